# revision 1
# baseline (speedup 1.0000x reference)
"""LocalScoreMachine Trainium2 kernel.

score[b,c,p] = -sum_n w[b,n,p]*(x[b,c,p]-m*I[n,c,p]) / (sig2 * sum_n w[b,n,p])
with w = exp(-box3(|x - m*I|^2 summed over c)/(2*sig2) - sub).

Expansion: box3(norm) = box3(A) + m^2*box3(S) - 2m*box3(z),
A = sum_c x_c^2 (b-only), S = sum_c I_c^2 (n-only), z = sum_c x_c*I_c.
The exp factor from box3(A) (and any per-(b,p) stabilizer) is constant in n,c
and cancels in the numerator/denominator ratio, so each core computes
    w' = exp(box3((m/sig2)*z - (m^2/(2 sig2))*S))
over its shard of N, accumulates SW = sum_n w', SWI_c = sum_n w'*I_c via
TensorE ones-matmuls, and the host combines 8 partial results:
    score = (m*SWI/SW - x)/sig2.

Sharding: dataset axis N=2048 -> 256 images per core (8 cores), as 2 tiles of
[128 partitions = n, (3,32,32) free].
"""

import sys

for _p in ("/opt/trn_rl_repo", "/opt/trn_rl_repo/concourse", "/opt/pypackages"):
    if _p not in sys.path:
        sys.path.append(_p)

from contextlib import ExitStack

import numpy as np

import concourse.bass as bass
import concourse.bacc as bacc
import concourse.mybir as mybir
import concourse.tile as tile
from concourse import bass_utils

B, N, C, H, W = 8, 2048, 3, 32, 32
P = H * W  # 1024 pixels
NCORES = 8
NLOC = N // NCORES  # 256
NT = NLOC // 128  # 2 partition tiles per core
F32 = mybir.dt.float32
AF = mybir.ActivationFunctionType

_cache = {}
_last_res = None


def _build(m: float, sig2: float):
    """Build + compile the per-core SPMD program. m, sig2 are compile-time."""
    nc = bacc.Bacc("TRN2", target_bir_lowering=False, debug=False)

    img_d = nc.dram_tensor("img", [NLOC, C, H, W], F32, kind="ExternalInput")
    xs_d = nc.dram_tensor("xs", [B, C * P], F32, kind="ExternalInput")
    out_d = nc.dram_tensor("out", [B, 4, P], F32, kind="ExternalOutput")

    c_s = -(m * m) / (2.0 * sig2)  # multiplies S
    # z scale m/sig2 is folded into xs on the host.

    with tile.TileContext(nc) as tc, ExitStack() as ctx:
        const = ctx.enter_context(tc.tile_pool(name="const", bufs=1))
        imgs = ctx.enter_context(tc.tile_pool(name="imgs", bufs=1))
        spool = ctx.enter_context(tc.tile_pool(name="spool", bufs=1))
        pre = ctx.enter_context(tc.tile_pool(name="pre", bufs=1))
        xrs_pool = ctx.enter_context(tc.tile_pool(name="xrs", bufs=4))
        workv = ctx.enter_context(tc.tile_pool(name="workv", bufs=2))
        work = ctx.enter_context(tc.tile_pool(name="work", bufs=3))
        psum = ctx.enter_context(
            tc.tile_pool(name="psum", bufs=2, space=bass.MemorySpace.PSUM)
        )
        acc_pool = ctx.enter_context(
            tc.tile_pool(name="acc", bufs=1, space=bass.MemorySpace.PSUM)
        )

        ones_row = const.tile([1, 128], F32)  # lhsT for broadcast (K=1,M=128)
        ones_col = const.tile([128, 32], F32)  # lhsT for reduction (K=128,M=32)
        nc.gpsimd.memset(ones_row[:], 1.0)
        nc.gpsimd.memset(ones_col[:], 1.0)



        img_ap = img_d.ap().rearrange("(t p) c h w -> t p (c h w)", p=128)
        itiles = []
        spp = []
        for t in range(NT):
            it = imgs.tile([128, C, P], F32, tag=f"img{t}", name=f"img{t}")
            nc.sync.dma_start(it[:], img_ap[t])
            itiles.append(it)

            # S'' = c_s * sum_c I_c^2
            sq = pre.tile([128, C, P], F32, tag="sq")
            nc.scalar.square(sq[:], it[:])
            s0 = work.tile([128, P], F32, tag="tmp")
            nc.vector.tensor_add(s0[:], sq[:, 0], sq[:, 1])
            s1 = work.tile([128, P], F32, tag="chain")
            nc.vector.tensor_add(s1[:], s0[:], sq[:, 2])
            sp = spool.tile([128, P], F32, tag=f"spp{t}", name=f"spp{t}")
            nc.vector.tensor_scalar_mul(sp[:], s1[:], c_s)
            spp.append(sp)

        for b in range(B):
            # stage xs[b] on partition 0, then broadcast via PE ones-matmul
            xsb = workv.tile([1, C * P], F32, tag="xsb", name=f"xsb_{b}")
            nc.sync.dma_start(xsb[:], xs_d.ap()[b][None, :])
            xrc = []
            for c in range(C):
                xp = psum.tile([128, P], F32, tag="xr", name=f"xr_{b}_{c}")
                for half in range(2):
                    nc.tensor.matmul(
                        xp[:, half * 512 : (half + 1) * 512],
                        ones_row[:],
                        xsb[0:1, c * P + half * 512 : c * P + half * 512 + 512],
                    )
                xr_sb = xrs_pool.tile([128, P], F32, tag="xrs", name=f"xrs_{b}_{c}")
                nc.scalar.copy(xr_sb[:], xp[:])
                xrc.append(xr_sb)

            # accumulators: quadrant-packed redundant-row [32,512] blocks
            # accq[half] rows: 0-31=SW, 32-63=SWI0, 64-95=SWI1; accr[half]=SWI2
            accq0 = acc_pool.tile([96, 512], F32, tag="accq0")
            accq1 = acc_pool.tile([96, 512], F32, tag="accq1")
            accr0 = acc_pool.tile([32, 512], F32, tag="accr0")
            accr1 = acc_pool.tile([32, 512], F32, tag="accr1")
            accq = [accq0, accq1]
            accr = [accr0, accr1]

            for t in range(NT):
                it = itiles[t]
                # u = S'' + sum_c I_c * xs_c   (xs pre-scaled by m/sig2)
                t0 = work.tile([128, P], F32, tag="tmp")
                nc.vector.tensor_mul(t0[:], it[:, 0], xrc[0][:])
                u0 = work.tile([128, P], F32, tag="chain")
                nc.vector.tensor_add(u0[:], t0[:], spp[t][:])
                t1 = work.tile([128, P], F32, tag="tmp")
                nc.vector.tensor_mul(t1[:], it[:, 1], xrc[1][:])
                u1 = work.tile([128, P], F32, tag="chain")
                nc.vector.tensor_add(u1[:], u0[:], t1[:])
                t2 = work.tile([128, P], F32, tag="tmp")
                nc.vector.tensor_mul(t2[:], it[:, 2], xrc[2][:])
                u = work.tile([128, H, W], F32, tag="chain")
                nc.vector.tensor_add(
                    u[:].rearrange("p h w -> p (h w)"), u1[:], t2[:]
                )

                # separable 3x3 box filter (zero pad), free dims (h, w)
                r = work.tile([128, H, W], F32, tag="tmp")  # t[w] = u[w]+u[w+1]
                nc.vector.tensor_add(r[:, :, 0:31], u[:, :, 0:31], u[:, :, 1:32])
                nc.scalar.copy(r[:, :, 31:32], u[:, :, 31:32])
                r2 = work.tile([128, H, W], F32, tag="chain")  # rowsum
                nc.vector.tensor_add(r2[:, :, 1:32], r[:, :, 1:32], u[:, :, 0:31])
                nc.scalar.copy(r2[:, :, 0:1], r[:, :, 0:1])

                s = work.tile([128, H, W], F32, tag="tmp")  # t2[h] = r2[h]+r2[h+1]
                nc.vector.tensor_add(s[:, 0:31, :], r2[:, 0:31, :], r2[:, 1:32, :])
                nc.scalar.copy(s[:, 31:32, :], r2[:, 31:32, :])
                arg = work.tile([128, H, W], F32, tag="chain")  # full box sum
                nc.vector.tensor_add(arg[:, 1:32, :], s[:, 1:32, :], r2[:, 0:31, :])
                nc.scalar.copy(arg[:, 0:1, :], s[:, 0:1, :])

                wt = work.tile([128, H, W], F32, tag="wt")
                nc.scalar.activation(wt[:], arg[:], AF.Exp)

                v = workv.tile([128, C, P], F32, tag="v")
                wflat = wt[:].rearrange("p h w -> p (h w)")
                for c in range(C):
                    nc.vector.tensor_mul(v[:, c], wflat, it[:, c])

                # reduce over n (partitions) via ones matmuls, accumulate in PSUM
                first, last = (t == 0), (t == NT - 1)
                for half in range(2):
                    sl = slice(half * 512, (half + 1) * 512)
                    nc.tensor.matmul(
                        accq[half][0:32], ones_col[:], wflat[:, sl],
                        start=first, stop=last,
                    )
                    nc.tensor.matmul(
                        accq[half][32:64], ones_col[:], v[:, 0, sl],
                        start=first, stop=last,
                    )
                    nc.tensor.matmul(
                        accq[half][64:96], ones_col[:], v[:, 1, sl],
                        start=first, stop=last,
                    )
                    nc.tensor.matmul(
                        accr[half][0:32], ones_col[:], v[:, 2, sl],
                        start=first, stop=last,
                    )

            for half in range(2):
                sl = slice(half * 512, (half + 1) * 512)
                oq = work.tile([96, 512], F32, tag="oq", name=f"oq_{b}_{half}")
                nc.scalar.copy(oq[:], accq[half][:])
                orr = work.tile([32, 512], F32, tag="orr", name=f"orr_{b}_{half}")
                nc.scalar.copy(orr[:], accr[half][:])
                nc.sync.dma_start(out_d.ap()[b, 0, sl], oq[0:1, :])
                nc.sync.dma_start(out_d.ap()[b, 1, sl], oq[32:33, :])
                nc.sync.dma_start(out_d.ap()[b, 2, sl], oq[64:65, :])
                nc.sync.dma_start(out_d.ap()[b, 3, sl], orr[0:1, :])

    nc.compile()
    return nc


def kernel(x, images, mu, sigma, t):
    x = np.ascontiguousarray(np.asarray(x, dtype=np.float32))
    images = np.ascontiguousarray(np.asarray(images, dtype=np.float32))
    m = float(np.asarray(mu)[int(t)])
    sig = float(np.asarray(sigma)[int(t)])
    sig2 = sig * sig

    key = (m, sig2)
    if key not in _cache:
        _cache[key] = _build(m, sig2)
    nc = _cache[key]

    xs = (x.reshape(B, C * P) * (m / sig2)).astype(np.float32)
    imgs = images.reshape(N, C * P)
    in_maps = []
    for k in range(NCORES):
        in_maps.append(
            {
                "img": np.ascontiguousarray(
                    imgs[k * NLOC : (k + 1) * NLOC].reshape(NLOC, C, H, W)
                ),
                "xs": xs,
            }
        )

    import os
    trace = bool(os.environ.get("KERNEL_TRACE"))
    res = bass_utils.run_bass_kernel_spmd(
        nc, in_maps, core_ids=list(range(NCORES)), trace=trace
    )
    global _last_res
    _last_res = res
    parts = np.stack([res.results[k]["out"] for k in range(NCORES)])  # [8,B,4,P]
    tot = parts.sum(axis=0)
    sw = tot[:, 0, :]  # [B,P]
    swi = tot[:, 1:4, :]  # [B,C,P]
    score = (m * swi / sw[:, None, :] - x.reshape(B, C, P)) / sig2
    return score.reshape(B, C, H, W).astype(np.float32)



# revision 7
# speedup vs baseline: 2.0197x; 2.0197x over previous
"""LocalScoreMachine Trainium2 kernel (pixel-partition layout).

score[b,c,p] = (m*SWI[b,c,p]/SW[b,p] - x[b,c,p]) / sig2
with w[b,n,p] = exp(box3((m/sig2)*z - (m^2/(2 sig2))*S)[b,n,p]),
z = sum_c x_c*I_c (pointwise over pixels), S = sum_c I_c^2,
SW = sum_n w, SWI_c = sum_n w*I_c.  The per-(b,p) stabilizer constant
cancels in the ratio, so no running max is needed.

Layout: partitions = pixels (8 chunks of 128 = 4 image rows x 32 cols),
free dim = n.  Sharding: 8 cores = 4 query-groups (2 queries each) x 2
dataset shards (1024 images each); host sums the two partial
(SW, SWI) shards per query.

Per chunk j (all tensors [128 pixels, ...] bf16 in SBUF):
  u[:,b,:] = (I0*s0 + St) + I1*s1 + I2*s2      3 fused STT ops/b (DVE)
             where s_c = x[b,c,pixel]*(m/sig2) is a per-partition scalar
             and St = -(m^2/(2 sig2))*S^T is host-precomputed.
  box3(u) via PE matmuls with banded 0/1 weights: T_box (9-band within
             the 4-row chunk) + T_prev/T_next (row coupling to adjacent
             chunks), PSUM-accumulated; zero padding falls out naturally.
  w = Exp(box_psum) on the Activation engine, accum_out -> SW.
  v_c = w * I_c via STT with accum_out -> SWI_c (c=0,1 on DVE, c=2 on
             GpSimd to balance engine load).
Outputs land in one [128, 8*b_loc*4] f32 tile -> single DMA out.
"""

import sys

for _p in ("/opt/trn_rl_repo", "/opt/trn_rl_repo/concourse", "/opt/pypackages"):
    if _p not in sys.path:
        sys.path.append(_p)

from contextlib import ExitStack

import numpy as np
import ml_dtypes

import concourse.bass as bass
import concourse.bacc as bacc
import concourse.mybir as mybir
import concourse.tile as tile
from concourse import bass_utils

B, N, C, H, W = 8, 2048, 3, 32, 32
P = H * W  # 1024 pixels
NCORES = 8
GB, GN = 4, 2  # query-groups x dataset shards
BLOC = B // GB  # 2 queries per core
NLOC = N // GN  # 1024 images per core
NCHUNK = P // 128  # 8 pixel chunks, each 4 image rows
ROWS = 128 // W  # 4 image rows per chunk
F32 = mybir.dt.float32
BF16 = mybir.dt.bfloat16
AF = mybir.ActivationFunctionType
MUL = mybir.AluOpType.mult
ADD = mybir.AluOpType.add
BF = ml_dtypes.bfloat16

_cache = {}
_last_res = None


def _box_mats():
    """Banded 0/1 matrices for box3 along the partition (pixel) axis.

    Partition k of a chunk is pixel (h, w) = (k // W, k % W) with h local
    to the chunk (0..ROWS-1).  matmul computes out[m,f] = sum_k T[k,m]*u[k,f].
    """
    k = np.arange(128)
    hk, wk = k // W, k % W
    hm, wm = hk[None, :], wk[None, :]
    hk, wk = hk[:, None], wk[:, None]
    wband = np.abs(wk - wm) <= 1
    tbox = ((np.abs(hk - hm) <= 1) & wband).astype(np.float32)
    tprev = ((hk == ROWS - 1) & (hm == 0) & wband).astype(np.float32)
    tnext = ((hk == 0) & (hm == ROWS - 1) & wband).astype(np.float32)
    return np.stack([tbox, tprev, tnext]).astype(BF)


def _build():
    nc = bacc.Bacc("TRN2", target_bir_lowering=False, debug=False)

    FIM = 4 * NLOC  # free size of one image-chunk tile: (ch, n), ch 3 = St
    its_d = nc.dram_tensor("its", [NCHUNK, 128, FIM], BF16, kind="ExternalInput")
    xt_d = nc.dram_tensor("xt", [NCHUNK, 128, BLOC * C], F32, kind="ExternalInput")
    t_d = nc.dram_tensor("tm", [3, 128, 128], BF16, kind="ExternalInput")
    # out free layout: (chunk, b, q) with q: 0..2 = SWI_c, 3 = SW
    out_d = nc.dram_tensor("out", [128, NCHUNK, BLOC, 4], F32, kind="ExternalOutput")

    NSL = (BLOC * NLOC * 4) // 2048  # 512-col psum slices per chunk (= 4)

    with tile.TileContext(nc) as tc, ExitStack() as ctx:
        const = ctx.enter_context(tc.tile_pool(name="const", bufs=1))
        itsp = ctx.enter_context(tc.tile_pool(name="its", bufs=3))
        upool = ctx.enter_context(tc.tile_pool(name="u", bufs=4))
        wpool = ctx.enter_context(tc.tile_pool(name="w", bufs=2))
        zwork = ctx.enter_context(tc.tile_pool(name="zw", bufs=4))
        vdve = ctx.enter_context(tc.tile_pool(name="vd", bufs=2))
        psum = ctx.enter_context(
            tc.tile_pool(name="ps", bufs=2, space=bass.MemorySpace.PSUM)
        )

        tmat = const.tile([128, 3, 128], BF16)
        nc.sync.dma_start(tmat[:], t_d.ap().rearrange("t k m -> k t m"))
        xt = const.tile([128, NCHUNK, BLOC, C], F32)
        nc.sync.dma_start(xt[:], xt_d.ap().rearrange("j p f -> p j f", f=BLOC * C))
        outsb = const.tile([128, NCHUNK, BLOC, 4], F32)

        its = []
        for j in range(NCHUNK):
            it = itsp.tile([128, 4, NLOC], BF16, tag="its", name=f"its{j}")
            nc.sync.dma_start(
                it[:].rearrange("p c n -> p (c n)"), its_d.ap()[j]
            )
            its.append(it)

        def make_u(j):
            u = upool.tile([128, BLOC, NLOC], BF16, tag="u", name=f"u{j}")
            for b in range(BLOC):
                q0 = zwork.tile([128, NLOC], BF16, tag="zw")
                nc.vector.scalar_tensor_tensor(
                    q0[:], its[j][:, 0], xt[:, j, b, 0:1], its[j][:, 3], MUL, ADD
                )
                q1 = zwork.tile([128, NLOC], BF16, tag="zw")
                nc.vector.scalar_tensor_tensor(
                    q1[:], its[j][:, 1], xt[:, j, b, 1:2], q0[:], MUL, ADD
                )
                nc.vector.scalar_tensor_tensor(
                    u[:, b], its[j][:, 2], xt[:, j, b, 2:3], q1[:], MUL, ADD
                )
            return u

        us = {0: make_u(0)}

        for j in range(NCHUNK):
            if j + 1 < NCHUNK:
                us[j + 1] = make_u(j + 1)
            # box3 over pixels: PSUM-accumulated banded matmuls
            bp = psum.tile([128, BLOC, NLOC], F32, tag="bp", name=f"bp{j}")
            bpf = bp[:].rearrange("p b n -> p (b n)")
            srcs = []
            if j > 0:
                srcs.append((1, us[j - 1]))
            srcs.append((0, us[j]))
            if j + 1 < NCHUNK:
                srcs.append((2, us[j + 1]))
            for s in range(NSL):
                sl = slice(s * 512, (s + 1) * 512)
                for i, (ti, ut) in enumerate(srcs):
                    nc.tensor.matmul(
                        bpf[:, sl],
                        tmat[:, ti],
                        ut[:].rearrange("p b n -> p (b n)")[:, sl],
                        start=(i == 0),
                        stop=(i == len(srcs) - 1),
                    )
            if j >= 2:
                del us[j - 2]

            w = wpool.tile([128, BLOC, NLOC], BF16, tag="w", name=f"w{j}")
            for b in range(BLOC):
                nc.scalar.activation(
                    w[:, b], bp[:, b], AF.Exp, accum_out=outsb[:, j, b, 3:4]
                )
            for b in range(BLOC):
                for c in range(C):
                    v = vdve.tile([128, NLOC], BF16, tag="vd")
                    nc.vector.scalar_tensor_tensor(
                        v[:], w[:, b], 1.0, its[j][:, c], MUL, MUL,
                        accum_out=outsb[:, j, b, c : c + 1],
                    )

        nc.sync.dma_start(out_d.ap(), outsb[:])

    nc.compile()
    return nc


def kernel(x, images, mu, sigma, t):
    x = np.ascontiguousarray(np.asarray(x, dtype=np.float32))
    images = np.asarray(images, dtype=np.float32)
    m = float(np.asarray(mu)[int(t)])
    sig = float(np.asarray(sigma)[int(t)])
    sig2 = sig * sig

    if "nc" not in _cache:
        _cache["nc"] = _build()
    nc = _cache["nc"]

    c_s = -(m * m) / (2.0 * sig2)
    # I^T: [P, C, N]; St = c_s * sum_c I_c^2 as channel 3
    it = images.transpose(2, 3, 1, 0).reshape(P, C, N)
    st = c_s * (it * it).sum(axis=1, keepdims=True)
    its_full = np.concatenate([it, st], axis=1)  # [P, 4, N] f32
    xs = (x * (m / sig2)).transpose(2, 3, 0, 1).reshape(P, B, C)  # [P, B, C]
    tmats = _box_mats()

    in_maps = []
    for k in range(NCORES):
        ib, in_ = k // GN, k % GN
        blo = ib * BLOC
        nsl = slice(in_ * NLOC, (in_ + 1) * NLOC)
        its_k = np.ascontiguousarray(
            its_full[:, :, nsl].reshape(NCHUNK, 128, 4 * NLOC).astype(BF)
        )
        xt_k = np.ascontiguousarray(
            xs[:, blo : blo + BLOC, :].reshape(NCHUNK, 128, BLOC * C)
        )
        in_maps.append({"its": its_k, "xt": xt_k, "tm": tmats})

    import os

    trace = bool(os.environ.get("KERNEL_TRACE"))
    res = bass_utils.run_bass_kernel_spmd(
        nc, in_maps, core_ids=list(range(NCORES)), trace=trace
    )
    global _last_res
    _last_res = res

    # parts[k]: [128, NCHUNK, BLOC, 4] -> per-core [P, BLOC, 4]
    sw = np.zeros((B, P), np.float32)
    swi = np.zeros((B, C, P), np.float32)
    for k in range(NCORES):
        ib = k // GN
        part = np.asarray(res.results[k]["out"], np.float32)
        part = part.transpose(1, 0, 2, 3).reshape(P, BLOC, 4)
        for bl in range(BLOC):
            b = ib * BLOC + bl
            sw[b] += part[:, bl, 3]
            swi[b] += part[:, bl, :3].T
    score = (m * swi / sw[:, None, :] - x.reshape(B, C, P)) / sig2
    return score.reshape(B, C, H, W).astype(np.float32)


# revision 11
# speedup vs baseline: 2.2833x; 1.1305x over previous
"""LocalScoreMachine Trainium2 kernel (pixel-partition layout, v2).

score[b,c,p] = (m*SWI[b,c,p]/SW[b,p] - x[b,c,p]) / sig2
with w[b,n,p] = exp(box3((m/sig2)*z - (m^2/(2 sig2))*S)[b,n,p]),
z = sum_c x_c*I_c (pointwise over pixels), S = sum_c I_c^2,
SW = sum_n w, SWI_c = sum_n w*I_c.  The per-(b,p) stabilizer constant
cancels in the SWI/SW ratio, so no running max is needed.

Layout: partitions = pixels (8 chunks of 128 = 4 image rows x 32 cols),
free dim = n.  Sharding: 8 cores = 4 query-groups (2 queries each) x 2
dataset shards (1024 images each); host sums the two partial
(SW, SWI) shards per query.

Engine mix per chunk j (chosen for the DVE perf-mode table: tensor_scalar
runs 4x in bf16, tensor_tensor 2x, fused scalar_tensor_tensor/TTR only 1x):
  p[b,c]   = I_c * s_c        6 tensor_scalar muls (DVE 4x), s_c = per-
                              partition scalar x[b,c,pixel]*(m/sig2)
  u[b]     = p0+p1+p2+St      3 b-batched tensor_tensor adds (DVE 2x)
  box3(u) via PE matmuls with banded 0/1 weights: T_box (9-band within the
           4-row chunk) + T_prev/T_next (row coupling to adjacent chunks),
           PSUM-accumulated; box3's zero padding falls out naturally.
  w[b]     = Exp(box_psum)    Activation engine, accum_out -> SW (free).
  SWI_c: either v_c = w*I_c (b-batched TT) + per-b act Copy-with-accum_out,
         or a per-b tensor_tensor_reduce on DVE - routed per (j,c) unit to
         balance DVE vs Act load.
Outputs land in one [128, chunk*b*4] f32 tile -> single DMA out.
"""

import sys

for _p in ("/opt/trn_rl_repo", "/opt/trn_rl_repo/concourse", "/opt/pypackages"):
    if _p not in sys.path:
        sys.path.append(_p)

from contextlib import ExitStack

import numpy as np
import ml_dtypes

import concourse.bass as bass
import concourse.bacc as bacc
import concourse.mybir as mybir
import concourse.tile as tile
from concourse import bass_utils

B, N, C, H, W = 8, 2048, 3, 32, 32
P = H * W  # 1024 pixels
NCORES = 8
GB, GN = 4, 2  # query-groups x dataset shards
BLOC = B // GB  # 2 queries per core
NLOC = N // GN  # 1024 images per core
NCHUNK = P // 128  # 8 pixel chunks, each 4 image rows
ROWS = 128 // W  # 4 image rows per chunk
F32 = mybir.dt.float32
BF16 = mybir.dt.bfloat16
AF = mybir.ActivationFunctionType
MUL = mybir.AluOpType.mult
ADD = mybir.AluOpType.add
BF = ml_dtypes.bfloat16

# SWI-reduction routing: unit (j, c) uses act Copy+accum when (j*C+c) is in
# the act set, else per-b TTR on DVE.  POOL_UADDS u-adds per chunk go to
# GpSimd (tensor_tensor is the one elementwise op the Pool engine accepts).
ACT_UNITS = 16
POOL_UADDS = 0

_cache = {}
_last_res = None


def _box_mats():
    """Banded 0/1 matrices for box3 along the partition (pixel) axis.

    Partition k of a chunk is pixel (h, w) = (k // W, k % W) with h local
    to the chunk (0..ROWS-1).  matmul computes out[m,f] = sum_k T[k,m]*u[k,f].
    """
    k = np.arange(128)
    hk, wk = k // W, k % W
    hm, wm = hk[None, :], wk[None, :]
    hk, wk = hk[:, None], wk[:, None]
    wband = np.abs(wk - wm) <= 1
    tbox = ((np.abs(hk - hm) <= 1) & wband).astype(np.float32)
    tprev = ((hk == ROWS - 1) & (hm == 0) & wband).astype(np.float32)
    tnext = ((hk == 0) & (hm == ROWS - 1) & wband).astype(np.float32)
    return np.stack([tbox, tprev, tnext]).astype(BF)


def _build():
    nc = bacc.Bacc("TRN2", target_bir_lowering=False, debug=False)

    FIM = 4 * NLOC  # free size of one image-chunk tile: (ch, n), ch 3 = St
    its_d = nc.dram_tensor("its", [NCHUNK, 128, FIM], BF16, kind="ExternalInput")
    xt_d = nc.dram_tensor("xt", [NCHUNK, 128, BLOC * C], F32, kind="ExternalInput")
    t_d = nc.dram_tensor("tm", [3, 128, 128], BF16, kind="ExternalInput")
    # out free layout: (chunk, b, q) with q: 0..2 = SWI_c, 3 = SW
    out_d = nc.dram_tensor("out", [128, NCHUNK, BLOC, 4], F32, kind="ExternalOutput")

    NSL = (BLOC * NLOC * 4) // 2048  # 512-col psum slices per chunk (= 4)

    with tile.TileContext(nc) as tc, ExitStack() as ctx:
        const = ctx.enter_context(tc.tile_pool(name="const", bufs=1))
        itsp = ctx.enter_context(tc.tile_pool(name="its", bufs=3))
        ppool = ctx.enter_context(tc.tile_pool(name="p", bufs=2))
        apool = ctx.enter_context(tc.tile_pool(name="a", bufs=4))
        upool = ctx.enter_context(tc.tile_pool(name="u", bufs=4))
        wpool = ctx.enter_context(tc.tile_pool(name="w", bufs=2))
        vpool = ctx.enter_context(tc.tile_pool(name="v", bufs=3))
        psum = ctx.enter_context(
            tc.tile_pool(name="ps", bufs=2, space=bass.MemorySpace.PSUM)
        )

        tmat = const.tile([128, 3, 128], BF16)
        nc.sync.dma_start(tmat[:], t_d.ap().rearrange("t k m -> k t m"))
        xt = const.tile([128, NCHUNK, BLOC, C], F32)
        nc.sync.dma_start(xt[:], xt_d.ap().rearrange("j p f -> p j f", f=BLOC * C))
        outsb = const.tile([128, NCHUNK, BLOC, 4], F32)

        its = []
        for j in range(NCHUNK):
            it = itsp.tile([128, 4, NLOC], BF16, tag="its", name=f"its{j}")
            nc.sync.dma_start(it[:].rearrange("p c n -> p (c n)"), its_d.ap()[j])
            its.append(it)

        def make_u(j):
            p = ppool.tile([128, BLOC, C, NLOC], BF16, tag="p")
            for b in range(BLOC):
                for c in range(C):
                    nc.vector.tensor_scalar_mul(
                        p[:, b, c], its[j][:, c], xt[:, j, b, c : c + 1]
                    )
            a1 = apool.tile([128, BLOC, NLOC], BF16, tag="a")
            a2 = apool.tile([128, BLOC, NLOC], BF16, tag="a")
            u = upool.tile([128, BLOC, NLOC], BF16, tag="u", name=f"u{j}")
            e1 = nc.gpsimd if POOL_UADDS >= 2 else nc.vector
            e2 = nc.gpsimd if POOL_UADDS >= 3 else nc.vector
            e3 = nc.gpsimd if POOL_UADDS >= 1 else nc.vector
            e1.tensor_add(a1[:], p[:, :, 0], p[:, :, 1])
            for b in range(BLOC):
                e2.tensor_add(a2[:, b], p[:, b, 2], its[j][:, 3])
            e3.tensor_add(u[:], a1[:], a2[:])
            return u

        us = {0: make_u(0)}

        for j in range(NCHUNK):
            if j + 1 < NCHUNK:
                us[j + 1] = make_u(j + 1)
            # box3 over pixels: PSUM-accumulated banded matmuls
            bp = psum.tile([128, BLOC, NLOC], F32, tag="bp", name=f"bp{j}")
            bpf = bp[:].rearrange("p b n -> p (b n)")
            srcs = []
            if j > 0:
                srcs.append((1, us[j - 1]))
            srcs.append((0, us[j]))
            if j + 1 < NCHUNK:
                srcs.append((2, us[j + 1]))
            for s in range(NSL):
                sl = slice(s * 512, (s + 1) * 512)
                for i, (ti, ut) in enumerate(srcs):
                    nc.tensor.matmul(
                        bpf[:, sl],
                        tmat[:, ti],
                        ut[:].rearrange("p b n -> p (b n)")[:, sl],
                        start=(i == 0),
                        stop=(i == len(srcs) - 1),
                    )
            if j >= 2:
                del us[j - 2]

            w = wpool.tile([128, BLOC, NLOC], BF16, tag="w", name=f"w{j}")
            for b in range(BLOC):
                nc.scalar.activation(
                    w[:, b], bp[:, b], AF.Exp, accum_out=outsb[:, j, b, 3:4]
                )
            for c in range(C):
                for b in range(BLOC):
                    v = vpool.tile([128, NLOC], BF16, tag="vr")
                    nc.vector.scalar_tensor_tensor(
                        v[:], w[:, b], 1.0, its[j][:, c], MUL, MUL,
                        accum_out=outsb[:, j, b, c : c + 1],
                    )

        nc.sync.dma_start(out_d.ap(), outsb[:])

    nc.compile()
    return nc


def kernel(x, images, mu, sigma, t):
    x = np.ascontiguousarray(np.asarray(x, dtype=np.float32))
    images = np.asarray(images, dtype=np.float32)
    m = float(np.asarray(mu)[int(t)])
    sig = float(np.asarray(sigma)[int(t)])
    sig2 = sig * sig

    if "nc" not in _cache:
        _cache["nc"] = _build()
    nc = _cache["nc"]

    c_s = -(m * m) / (2.0 * sig2)
    # I^T: [P, C, N]; St = c_s * sum_c I_c^2 as channel 3
    it = images.transpose(2, 3, 1, 0).reshape(P, C, N)
    st = c_s * (it * it).sum(axis=1, keepdims=True)
    its_full = np.concatenate([it, st], axis=1)  # [P, 4, N] f32
    xs = (x * (m / sig2)).transpose(2, 3, 0, 1).reshape(P, B, C)  # [P, B, C]
    tmats = _box_mats()

    in_maps = []
    for k in range(NCORES):
        ib, in_ = k // GN, k % GN
        blo = ib * BLOC
        nsl = slice(in_ * NLOC, (in_ + 1) * NLOC)
        its_k = np.ascontiguousarray(
            its_full[:, :, nsl].reshape(NCHUNK, 128, 4 * NLOC).astype(BF)
        )
        xt_k = np.ascontiguousarray(
            xs[:, blo : blo + BLOC, :].reshape(NCHUNK, 128, BLOC * C)
        )
        in_maps.append({"its": its_k, "xt": xt_k, "tm": tmats})

    import os

    trace = bool(os.environ.get("KERNEL_TRACE"))
    res = bass_utils.run_bass_kernel_spmd(
        nc, in_maps, core_ids=list(range(NCORES)), trace=trace
    )
    global _last_res
    _last_res = res

    # parts[k]: [128, NCHUNK, BLOC, 4] -> per-core [P, BLOC, 4]
    sw = np.zeros((B, P), np.float32)
    swi = np.zeros((B, C, P), np.float32)
    for k in range(NCORES):
        ib = k // GN
        part = np.asarray(res.results[k]["out"], np.float32)
        part = part.transpose(1, 0, 2, 3).reshape(P, BLOC, 4)
        for bl in range(BLOC):
            b = ib * BLOC + bl
            sw[b] += part[:, bl, 3]
            swi[b] += part[:, bl, :3].T
    score = (m * swi / sw[:, None, :] - x.reshape(B, C, P)) / sig2
    return score.reshape(B, C, H, W).astype(np.float32)
